# revision 120
# baseline (speedup 1.0000x reference)
"""BiAttention (binary attention transformer block) Trainium2 kernel.

Forward-pass reduction of the reference:
  - softmax cancels:  stop_gradient(binq - soft) + soft == binq  (forward)
  - sign() is invariant to the positive per-row qkv weight scale
So per batch element (one per NeuronCore, 8 cores data-parallel):
  bq,bk,bv = sign(x @ sign(Wqkv).T)   split into heads
  A        = (bq @ bk.T > 0)          in {0,1}
  oo       = A @ bv                   exact small integers
  out      = (oo @ sign(Wproj).T) * mean(|Wproj|,axis=1) + b_proj

Performance structure (PE cycles ~270k vs 430k for the naive version):
  - QKV at 1.25 passes: fp16 hi pass (1.0 cyc/row) + fp8e5m2 residual
    pass under DoubleRow (0.5 cyc/row, 3 instrs).  e5m2 reaches the
    2^-12-scaled residuals that e4m3 flushes to zero; end-to-end rel err
    ~9.2e-3 vs the 2e-2 gate (sign flips in the binarized pipeline are
    the error mechanism, so single-pass fp16/f32r both fail).
  - scores under DoubleRow: q/k fp8 tiles partition-folded [128,N] ->
    [64,2,N] by one DMA (channel map s=2p+j, identical for q and k so
    dot products are preserved), contracting 32x2 at 0.5 cyc/row.
  - A@V fp8 DoubleRow over 256-token chunks; proj in fp16 with the
    mean|Wproj| scale folded into the transposed weights (host-computed
    sc2 input) so the evac is a single bias add.
  - w_qkv/w_proj transposes ride the xbar DMA (sign -> fp16 ->
    dma_start_transpose); x transposes stay on the PE where they fill
    the load-phase bubble; score binarize rotates ACT(sigmoid trick)/DVE
    (Pool cannot touch PSUM in the neuronx codegen).
"""

import numpy as np

import concourse.bacc as bacc
import concourse.bass as bass
import concourse.mybir as mybir
import concourse.tile as tile
from concourse.masks import make_identity

FP32 = mybir.dt.float32
FP16 = mybir.dt.float16
FP8 = mybir.dt.float8e4
FP8E5 = mybir.dt.float8e5
AF = mybir.ActivationFunctionType
ALU = mybir.AluOpType
DR = mybir.MatmulPerfMode.DoubleRow

B, N, C = 8, 1024, 768
H, D = 12, 64
C3 = 3 * C  # 2304
NK = C // 128  # 6 contraction chunks
NM = N // 128  # 8 token chunks
NOC = C3 // 128  # 18 qkv output chunks


QKV_MODE = "hie5"  # "hie5" (fp16 hi + fp8e5 DoubleRow lo), "hilo" (fp16 2-pass), "f32r"

# at-binarize rotation: ACT/DVE only (Pool cannot touch PSUM in the
# neuronx codegen); DVE slightly favored since ACT carries more evacs
BIN_PATTERN = "DADADADDADADADDA"
WT_MODE = "xbar"  # "pe" (transpose f32 on PE, sign on evac) or "xbar" (sign then DMA-transpose)


def build_nc(repeat=1):
    nc = bacc.Bacc("TRN2", target_bir_lowering=False, debug=True)

    x_d = nc.dram_tensor("x", [N, C], FP32, kind="ExternalInput")
    wqkv_d = nc.dram_tensor("w_qkv", [C3, C], FP32, kind="ExternalInput")
    wproj_d = nc.dram_tensor("w_proj", [C, C], FP32, kind="ExternalInput")
    bproj_d = nc.dram_tensor("b_proj", [1, C], FP32, kind="ExternalInput")
    sc2_d = nc.dram_tensor("sc2", [128, NK], FP32, kind="ExternalInput")
    out_d = nc.dram_tensor("out", [N, C], FP32, kind="ExternalOutput")

    # DRAM views: row r = chunk*128 + partition
    x_v = x_d[:].rearrange("(c p) f -> p c f", p=128)  # [128, 8, 768]
    wqkv_v = wqkv_d[:].rearrange("(c p) f -> p c f", p=128)  # [128, 18, 768]
    wproj_v = wproj_d[:].rearrange("(c p) f -> p c f", p=128)  # [128, 6, 768]
    out_v = out_d[:].rearrange("(c p) f -> p c f", p=128)  # [128, 8, 768]

    with tile.TileContext(nc) as tc:
        for _rep in range(repeat):
            _emit_body(nc, tc, _rep, x_v, wqkv_v, wproj_v, bproj_d, sc2_d, out_v)

    nc.compile()
    return nc


def _emit_body(nc, tc, rep, x_v, wqkv_v, wproj_v, bproj_d, sc2_d, out_v):
    _p = f"r{rep}_"
    if True:
        with (
            tc.tile_pool(name=_p + "persist", bufs=1) as pp,
            tc.tile_pool(name=_p + "stage", bufs=3) as sp,
            tc.tile_pool(name=_p + "xstage", bufs=2) as xsp,
            tc.tile_pool(name=_p + "wstage", bufs=3) as wp,
            tc.tile_pool(name=_p + "qk", bufs=5) as qkp,
            tc.tile_pool(name=_p + "at", bufs=4) as atp,
            tc.tile_pool(name=_p + "outstage", bufs=2) as op,
            tc.tile_pool(name=_p + "w2pre", bufs=2) as w2p,
        ):
            # ---- persistent SBUF ----
            FPR = mybir.dt.float32r
            if QKV_MODE == "hie5":
                # hi = fp16(x); lo = x - hi in fp8e5m2 (range reaches the
                # 2^-12-scaled residuals; e4m3 would flush them to zero).
                # lo pass runs DoubleRow at 0.5 cycles/row.
                xT_hi = pp.tile([128, NK, N], FP16, tag="xT_hi")  # [c%128, c//128, n]
                xT_lo8 = pp.tile([128, NK, N], FP8E5, tag="xT_lo8")
                wsT = pp.tile([128, NK, C3], FP16, tag="wsT")  # sign(wqkv).T
                ws8 = pp.tile([128, NK, C3], FP8E5, tag="ws8")
                qkv_srcs = (xT_hi,)
            elif QKV_MODE == "hilo":
                xT_hi = pp.tile([128, NK, N], FP16, tag="xT_hi")  # [c%128, c//128, n]
                xT_lo = pp.tile([128, NK, N], FP16, tag="xT_lo")
                wsT = pp.tile([128, NK, C3], FP16, tag="wsT")  # sign(wqkv).T
                qkv_srcs = (xT_hi, xT_lo)
            else:
                xT_r = pp.tile([128, NK, N], FPR, tag="xT_r")
                wsT = pp.tile([128, NK, C3], FPR, tag="wsT")
                qkv_srcs = (xT_r,)
            w2T = pp.tile([128, NK, C], FP16, tag="w2T")  # sign(wproj).T
            v_nat = pp.tile([128, NM, C], FP8, tag="v_nat")  # v, ±0.5, [m%128, m//128, hd]
            ooT = pp.tile([128, NK, N], FP16, tag="ooT")  # attn out transposed
            bias_row = pp.tile([1, C], FP32, tag="bias_row")
            bias_rep = pp.tile([128, C], FP32, tag="bias_rep")
            ident = pp.tile([128, 128], FP32, tag="ident")

            sigb = pp.tile([128, 1], FP32, tag="sigb")
            nc.gpsimd.memset(sigb[:], -32.0)
            make_identity(nc, ident[:])

            # ---- prep phase: loads + transposes ----
            tr_pool_cm = tc.tile_pool(name=_p + "ps_tr", bufs=2, space="PSUM")
            ps_tr = tr_pool_cm.__enter__()

            # Load issue order: x chunk 0 first (gates the first PE
            # transpose), v-weight chunks early (gate the sign->xbar chain
            # feeding the v-part matmuls), remaining x, then q/k weights.
            # Batched DMAs cut SP issue serialization.
            x_stages = {}

            def load_x(c0, nb):
                t = xsp.tile([128, nb, C], FP32, tag=f"x_stage{nb}", name=f"xs{c0}")
                nc.sync.dma_start(t[:], x_v[:, c0 : c0 + nb, :])
                for j in range(nb):
                    x_stages[c0 + j] = (t, j)

            w_stages = {}

            def load_w(oc0, nb):
                t = wp.tile([128, nb, C], FP32, tag="w_stage", name=f"wqs{oc0}")
                nc.sync.dma_start(t[:], wqkv_v[:, oc0 : oc0 + nb, :])
                for j in range(nb):
                    w_stages[oc0 + j] = (t, j)

            for oc in range(12, 15):
                load_w(oc, 1)
            load_x(0, 1)
            for oc in range(15, 18):
                load_w(oc, 1)
            load_x(1, 1)
            load_x(2, 2)
            load_x(4, 2)
            load_x(6, 2)


            # x: transpose on PE, split into fp16 hi + fp8e5 lo
            def process_x(cc):
                xs_t, xs_j = x_stages[cc]
                xs = xs_t[:, xs_j, :]
                xtp = ps_tr.tile([128, C], FP32, tag="tr_ps", name=f"xtr{cc}")
                for k in range(NK):
                    nc.tensor.transpose(
                        xtp[:, k * 128 : (k + 1) * 128],
                        xs[:, k * 128 : (k + 1) * 128],
                        ident[:],
                    )
                if QKV_MODE == "hie5":
                    dst_hi = xT_hi[:, :, cc * 128 : (cc + 1) * 128]
                    dst_lo = xT_lo8[:, :, cc * 128 : (cc + 1) * 128]
                    nc.vector.tensor_scalar(dst_hi, xtp[:], 1.0, None, ALU.mult)
                    nc.vector.tensor_tensor(dst_lo, xtp[:], dst_hi, ALU.subtract)
                elif QKV_MODE == "hilo":
                    dst_hi = xT_hi[:, :, cc * 128 : (cc + 1) * 128]
                    dst_lo = xT_lo[:, :, cc * 128 : (cc + 1) * 128]
                    nc.scalar.activation(dst_hi, xtp[:], AF.Copy)
                    nc.vector.tensor_tensor(dst_lo, xtp[:], dst_hi, ALU.subtract)
                else:
                    nc.scalar.activation(
                        xT_r[:, :, cc * 128 : (cc + 1) * 128], xtp[:], AF.Copy
                    )

            # w_proj: sign+transpose; |.| row-means via accum  (emitted after
            # the w_qkv/v-part phase: its results are only needed by proj)
            def emit_w2_prep():
              nc.sync.dma_start(bias_row[:], bproj_d[:])
              sc2c = sp.tile([128, NK], FP32, tag="sc2c")
              nc.sync.dma_start(sc2c[:], sc2_d[:])
              nc.gpsimd.partition_broadcast(bias_rep[:], bias_row[:])
              for cc in range(NK):
                w2s = w2p.tile([128, C], FP32, tag="w2_stage", name=f"w2s{cc}")
                nc.sync.dma_start(w2s[:], wproj_v[:, cc, :])
                # sign then fold the per-output-row sc2 scale in (fp16,
                # ~2^-11 rel: negligible vs the e5m2 lo-pass error), so the
                # proj evac needs no separate scale multiply
                w2sg = sp.tile([128, C], FP16, tag="w2_sign", name=f"w2sg{cc}")
                nc.scalar.activation(w2sg[:], w2s[:], AF.Sign)
                nc.vector.tensor_scalar(
                    w2sg[:], w2sg[:], sc2c[:, cc : cc + 1], None, ALU.mult
                )
                nc.sync.dma_start_transpose(
                    w2T[:, :, cc * 128 : (cc + 1) * 128], w2sg[:]
                )
            # w_qkv: sign -> fp16 wsT -> fp8e5 ws8 (v chunks first)
            vpart_emitted = False
            vp_cm = None

            def emit_v_m(m):
                    for half in range(2):
                        vp = ps_v.tile([128, 384], FP32, tag="v_ps", name=f"vps{m}_{half}")
                        vsl = slice(1536 + half * 384, 1536 + (half + 1) * 384)
                        if QKV_MODE == "hie5":
                            for k in range(NK):
                                nc.tensor.matmul(
                                    vp[:],
                                    lhsT=xT_hi[:, k, m * 128 : (m + 1) * 128],
                                    rhs=wsT[:, k, vsl],
                                    start=(k == 0),
                                    stop=False,
                                )
                            for kk in range(NK // 2):
                                nc.tensor.matmul(
                                    vp[:],
                                    lhsT=xT_lo8[:, 2 * kk : 2 * kk + 2, m * 128 : (m + 1) * 128],
                                    rhs=ws8[:, 2 * kk : 2 * kk + 2, vsl],
                                    perf_mode=DR,
                                    start=False,
                                    stop=(kk == NK // 2 - 1),
                                )
                        else:
                            ns = len(qkv_srcs)
                            for k in range(NK):
                                for si, src in enumerate(qkv_srcs):
                                    nc.tensor.matmul(
                                        vp[:],
                                        lhsT=src[:, k, m * 128 : (m + 1) * 128],
                                        rhs=wsT[:, k, vsl],
                                        start=(k == 0 and si == 0),
                                        stop=(k == NK - 1 and si == ns - 1),
                                    )
                        # v as +-1 (Sign): one ACT op; A@(+-1) == 2*(A@(+-0.5))
                        nc.scalar.activation(
                            v_nat[:, m, half * 384 : (half + 1) * 384],
                            vp[:],
                            AF.Sign,
                        )

            def process_w(oc, wt_mode=None):
                ws_t, ws_j = w_stages[oc]
                ws = ws_t[:, ws_j, :]
                if (wt_mode or WT_MODE) == "pe" or QKV_MODE == "f32r":
                    wtp = ps_tr.tile([128, C], FP32, tag="tr_ps", name=f"wtr{oc}")
                    for k in range(NK):
                        nc.tensor.transpose(
                            wtp[:, k * 128 : (k + 1) * 128],
                            ws[:, k * 128 : (k + 1) * 128],
                            ident[:],
                        )
                    nc.scalar.activation(
                        wsT[:, :, oc * 128 : (oc + 1) * 128], wtp[:], AF.Sign
                    )
                else:
                    wsg = wp.tile([128, C], FP16, tag="w_sign")
                    nc.scalar.activation(wsg[:], ws[:], AF.Sign)
                    nc.sync.dma_start_transpose(
                        wsT[:, :, oc * 128 : (oc + 1) * 128], wsg[:]
                    )
                if QKV_MODE == "hie5":
                    nc.gpsimd.tensor_copy(
                        ws8[:, :, oc * 128 : (oc + 1) * 128],
                        wsT[:, :, oc * 128 : (oc + 1) * 128],
                    )

            # v chunks transpose on PE first (their loads land before x),
            # then per-m: transpose x chunk m and immediately run the v-part
            # matmuls for m — the PE stays fed instead of chasing the
            # serialized DMA queue.
            for oc in range(12, 18):
                process_w(oc, wt_mode="pe")
            vp_cm = tc.tile_pool(name=_p + "ps_v", bufs=2, space="PSUM")
            ps_v = vp_cm.__enter__()
            for m in range(NM):
                process_x(m)
                emit_v_m(m)
            vpart_emitted = True
            # q/k weights: pair loads interleaved with sign+xbar processing so
            # each xbar transpose's SP issue precedes the next loads' issues
            for pair_q, pair_k in ((0, 6), (2, 8), (4, 10)):
                load_w(pair_q, 2)
                load_w(pair_k, 2)
                for oc in (pair_q, pair_k, pair_q + 1, pair_k + 1):
                    process_w(oc)
            assert vpart_emitted
            vp_cm.__exit__(None, None, None)
            tr_pool_cm.__exit__(None, None, None)

            # ---- per head-pair: q/k chunks, scores, binarize, A@V ----
            hp_psum_cms = [
                tc.tile_pool(name=_p + "ps_qk", bufs=2, space="PSUM"),
                tc.tile_pool(name=_p + "ps_s", bufs=2, space="PSUM"),
                tc.tile_pool(name=_p + "ps_oo", bufs=2, space="PSUM"),
            ]
            ps_qk, ps_s, ps_oo = [cm.__enter__() for cm in hp_psum_cms]
            bin_idx = 0
            qkTs = {}

            def emit_qk(hp):
                qkT = {}
                for role, oc in (("q", hp), ("k", 6 + hp)):
                    t = qkp.tile([128, N], FP8, tag="qkT", bufs=2, name=f"qkT_{role}{hp}")
                    for ncol in range(2):
                        qp = ps_qk.tile([128, 512], FP32, tag="qk_ps")
                        nsl = slice(ncol * 512, (ncol + 1) * 512)
                        ocsl = slice(oc * 128, (oc + 1) * 128)
                        if QKV_MODE == "hie5":
                            for k in range(NK):
                                nc.tensor.matmul(
                                    qp[:],
                                    lhsT=wsT[:, k, ocsl],
                                    rhs=xT_hi[:, k, nsl],
                                    start=(k == 0),
                                    stop=False,
                                )
                            for kk in range(NK // 2):
                                nc.tensor.matmul(
                                    qp[:],
                                    lhsT=ws8[:, 2 * kk : 2 * kk + 2, ocsl],
                                    rhs=xT_lo8[:, 2 * kk : 2 * kk + 2, nsl],
                                    perf_mode=DR,
                                    start=False,
                                    stop=(kk == NK // 2 - 1),
                                )
                        else:
                            ns = len(qkv_srcs)
                            for k in range(NK):
                                for si, src in enumerate(qkv_srcs):
                                    nc.tensor.matmul(
                                        qp[:],
                                        lhsT=wsT[:, k, ocsl],
                                        rhs=src[:, k, nsl],
                                        start=(k == 0 and si == 0),
                                        stop=(k == NK - 1 and si == ns - 1),
                                    )
                        nc.scalar.activation(
                            t[:, ncol * 512 : (ncol + 1) * 512], qp[:], AF.Sign
                        )
                    # partition fold [128, N] -> [64, 2, N] (channel s = 2p+j,
                    # same mapping for q and k so the head dot products are
                    # preserved): scores then run fp8 DoubleRow at 0.5 cyc/row
                    f = qkp.tile([64, 2, N], FP8, tag="qkF", bufs=4, name=f"qkF_{role}{hp}")
                    nc.sync.dma_start(f[:], t[:])
                    qkT[role] = f
                qkTs[hp] = qkT

            emit_qk(0)
            emit_w2_prep()
            for hp in range(6):
                qkT = qkTs.pop(hp)
                at = {}
                for h01 in range(2):
                    at[h01] = atp.tile([128, NM, N], FP8, tag="at", name=f"at{hp}_{h01}")
                for m in range(NM):
                    for h01 in range(2):
                        ph = 32 * h01
                        for ncol in range(2):
                            sp_ps = ps_s.tile(
                                [128, 512], FP32, tag="s_ps", bufs=4,
                                name=f"sps{hp}_{m}_{h01}_{ncol}",
                            )
                            nc.tensor.matmul(
                                sp_ps[:],
                                lhsT=qkT["k"][ph : ph + 32, :, m * 128 : (m + 1) * 128],
                                rhs=qkT["q"][ph : ph + 32, :, ncol * 512 : (ncol + 1) * 512],
                                perf_mode=DR,
                            )
                            dst = at[h01][:, m, ncol * 512 : (ncol + 1) * 512]
                            eng = BIN_PATTERN[bin_idx % len(BIN_PATTERN)]
                            if eng == "A":
                                nc.scalar.activation(
                                    dst, sp_ps[:], AF.Sigmoid, bias=sigb[:], scale=32.0
                                )
                            elif eng == "D":
                                nc.vector.tensor_scalar(
                                    dst, sp_ps[:], 0.0, None, ALU.is_gt
                                )
                            else:
                                nc.vector.tensor_scalar(
                                    dst, sp_ps[:], 0.0, None, ALU.is_gt
                                )
                            bin_idx += 1

                if hp + 1 < 6:
                    emit_qk(hp + 1)

                for h01 in range(2):
                    h = 2 * hp + h01
                    if h01 == 1:
                        oo_tmp = op.tile([64, N], FP16, tag="oo_tmp", name=f"oo_tmp{hp}")
                    for ncol in range(2):
                        oo_ps = ps_oo.tile(
                            [64, 512], FP32, tag="oo_ps", name=f"oo_ps{hp}_{h01}_{ncol}"
                        )
                        for j in range(4):
                            nc.tensor.matmul(
                                oo_ps[:],
                                lhsT=v_nat[:, 2 * j : 2 * j + 2, h * 64 : (h + 1) * 64],
                                rhs=at[h01][:, 2 * j : 2 * j + 2, ncol * 512 : (ncol + 1) * 512],
                                perf_mode=DR,
                                start=(j == 0),
                                stop=(j == 3),
                            )
                        # odd head's lanes must land on partitions 64-127: evac
                        # to a temp then partition-shift with a small DMA
                        dsth = ooT[0:64, hp, :] if h01 == 0 else oo_tmp[:]
                        csl = dsth[:, ncol * 512 : (ncol + 1) * 512]
                        if ncol == 0:
                            nc.scalar.activation(csl, oo_ps[:], AF.Copy)
                        else:
                            nc.vector.tensor_scalar(csl, oo_ps[:], 1.0, None, ALU.mult)
                    if h01 == 1:
                        nc.sync.dma_start(ooT[64:128, hp, :], oo_tmp[:])
            for cm in reversed(hp_psum_cms):
                cm.__exit__(None, None, None)

        # ---- projection ----
        with (
            tc.tile_pool(name=_p + "proj_out", bufs=3) as pop,
            tc.tile_pool(name=_p + "ps_proj", bufs=2, space="PSUM") as ps_p,
        ):
            for m in range(NM):
                ot = pop.tile([128, C], FP32, tag="out_stage")
                for n0, nw in ((0, 512), (512, 256)):
                    pps = ps_p.tile([128, nw], FP32, tag=f"p_ps{n0}")
                    for k in range(NK):
                        nc.tensor.matmul(
                            pps[:],
                            lhsT=ooT[:, k, m * 128 : (m + 1) * 128],
                            rhs=w2T[:, k, n0 : n0 + nw],
                            start=(k == 0),
                            stop=(k == NK - 1),
                        )
                    nc.vector.tensor_tensor(
                        ot[:, n0 : n0 + nw],
                        pps[:],
                        bias_rep[:, n0 : n0 + nw],
                        ALU.add,
                    )
                nc.sync.dma_start(out_v[:, m, :], ot[:])


_CACHE = {}


def _get_exec():
    """Build (once) and cache a jitted SPMD executable for the 8-core kernel."""
    if "exec" in _CACHE:
        return _CACHE["exec"]
    import jax
    import concourse.mybir as _mybir
    from jax.sharding import Mesh, PartitionSpec
    from jax.experimental.shard_map import shard_map
    from concourse.bass2jax import _bass_exec_p, install_neuronx_cc_hook

    nc = build_nc()
    install_neuronx_cc_hook()

    in_names, out_names, out_avals = [], [], []
    for alloc in nc.m.functions[0].allocations:
        if not isinstance(alloc, _mybir.MemoryLocationSet):
            continue
        name = alloc.memorylocations[0].name
        if alloc.kind == "ExternalInput":
            if name not in ("dbg_addr", "partition_id"):
                in_names.append(name)
        elif alloc.kind == "ExternalOutput":
            out_names.append(name)
            out_avals.append(
                jax.core.ShapedArray(tuple(alloc.tensor_shape), _mybir.dt.np(alloc.dtype))
            )
    if nc.dbg_addr is not None:
        in_names.append(nc.dbg_addr.name)
    n_params = len(in_names)
    n_outs = len(out_names)
    partition_name = nc.partition_id_tensor.name if nc.partition_id_tensor else None
    all_in_names = tuple(
        in_names + out_names + ([partition_name] if partition_name else [])
    )
    donate = tuple(range(n_params, n_params + n_outs))

    def _body(*args):
        operands = list(args)
        if partition_name is not None:
            from concourse.bass2jax import partition_id_tensor

            operands.append(partition_id_tensor())
        outs = _bass_exec_p.bind(
            *operands,
            out_avals=tuple(out_avals),
            in_names=all_in_names,
            out_names=tuple(out_names),
            lowering_input_output_aliases=(),
            sim_require_finite=True,
            sim_require_nnan=True,
            nc=nc,
        )
        return tuple(outs)

    devices = jax.devices()[:B]
    mesh = Mesh(np.array(devices), ("core",))
    in_specs = (PartitionSpec("core"),) * (n_params + n_outs)
    out_specs = (PartitionSpec("core"),) * n_outs
    sharded = jax.jit(
        shard_map(_body, mesh=mesh, in_specs=in_specs, out_specs=out_specs, check_rep=False),
        donate_argnums=donate,
        keep_unused=True,
    )
    _CACHE["exec"] = (sharded, in_names, out_names, out_avals, mesh)
    return _CACHE["exec"]


def _concat_inputs(x, w_qkv, w_proj, b_proj):
    """Per-core inputs concatenated along axis 0 (shard_map convention)."""
    x = np.asarray(x, np.float32)
    w_qkv = np.asarray(w_qkv, np.float32)
    w_proj = np.asarray(w_proj, np.float32)
    b_proj = np.asarray(b_proj, np.float32).reshape(1, C)
    sc2 = np.ascontiguousarray(
        np.mean(np.abs(w_proj), axis=1).astype(np.float32).reshape(NK, 128).T
    )
    per_core = {
        "x": [np.ascontiguousarray(x[b]) for b in range(B)],
        "w_qkv": [w_qkv] * B,
        "w_proj": [w_proj] * B,
        "b_proj": [b_proj] * B,
        "sc2": [sc2] * B,
        "dbg_addr": [np.zeros((1, 2), np.uint32)] * B,
    }
    return per_core


def _zero_outs(out_names, out_avals):
    return [
        np.zeros((B * a.shape[0], *a.shape[1:]), a.dtype) for a in out_avals
    ]


def kernel(x, w_qkv, w_proj, b_proj):
    sharded, in_names, out_names, out_avals, mesh = _get_exec()
    per_core = _concat_inputs(x, w_qkv, w_proj, b_proj)
    concat_in = [np.concatenate(per_core[name], axis=0) for name in in_names]
    out_arrs = sharded(*concat_in, *_zero_outs(out_names, out_avals))
    i = out_names.index("out")
    a = out_avals[i]
    return np.asarray(out_arrs[i]).reshape(B, *a.shape)

